# revision 14
# baseline (speedup 1.0000x reference)
"""Trainium2 Bass kernel for DecoderMultiHeadAttention, sharded over 8 cores.

Sharding: core c handles batch b=c//4 and head-group g=c%4 (4 of 16 heads).
Each core computes q/k/v projections for its heads, masked softmax attention
(transpose-free: logits computed as logitsT[j,i] so the softmax reduction is
along the matmul contraction), and a partial W_o projection summed over its
heads. Host sums the 4 partial outputs per batch.

v3 pipeline (emission order == engine execution order):
  B (q/k proj, fp16)  ->  fused loop: logits/exp/mask(hp0,ib0) + C(v proj)
  ->  7 attention blocks with: PV of the previous block's exp tiles where
      lagged, softmax normalization of the previous block deferred into
      jt==2 of the current block (reciprocal broadcast via a 1-row PE
      matmul from partition 64 -- no DRAM round trip, so the in-order DVE
      queue never waits on DMA latency), W_o blocks of the previous
      (hp==1) i-block split into jt==4/jt==9 slots, and next i-block mask
      chunk DMAs spread one per jt.
Everything on the PE is fp16 (fp32r HIGH-power mode triggered a ~50%
hardware duty-cycle throttle); host converts inputs to fp16 and sums the
fp16 partial outputs.

Self-contained: hardcodes all shapes from the problem spec.
"""

import os
import sys
import types

import numpy as np

# ---------------------------------------------------------------------------
# Environment shims (axon NTFF hook registry + no-op artifact upload)
# ---------------------------------------------------------------------------


def _install_shims():
    if "antenv.axon_hooks" not in sys.modules:
        mod = types.ModuleType("antenv.axon_hooks")
        _hook = [None]
        mod.set_axon_ntff_profile_hook = lambda h: _hook.__setitem__(0, h)
        mod.get_axon_ntff_profile_hook = lambda: _hook[0]
        sys.modules["antenv.axon_hooks"] = mod
        try:
            import antenv

            antenv.axon_hooks = mod
        except Exception:
            pass
        try:
            from trn_agent_boot.trn_boot import _ntff_profile_via_ctypes

            mod.set_axon_ntff_profile_hook(
                _ntff_profile_via_ctypes("/opt/axon/libaxon_pjrt.so")
            )
        except Exception:
            pass
    try:
        import concourse.bass_utils as bass_utils

        bass_utils.upload_artifacts = lambda tmpdir: f"file://{tmpdir}"
    except Exception:
        pass


_install_shims()

from contextlib import ExitStack

import concourse.bass as bass
import concourse.tile as tile
from concourse import bacc, mybir
import concourse.bass_utils as bass_utils

f32 = mybir.dt.float32
f16 = mybir.dt.float16

# Problem constants
N_BATCH = 2
S = 2048
E = 1024
H = 16
HD = 64
NC = 8
G = 4  # head groups (one per core within a batch)
NEG_SCALE = 0.125  # 1/sqrt(hd)
EXP_SHIFT = -5.0  # constant pre-exp shift so fp16 exp can't overflow
# (cancels exactly in the softmax normalization; logit/8 max is ~13.9
# for this problem's input distribution, fp16 exp overflows at 11.09)

JT = S // 128  # 16 j-tiles (key index)
IB = S // 512  # 4 i-blocks (query index)
ES = E // 128  # 8 E slices
ET = E // 128  # 8 output-channel tiles


def build_nc():
    nc = bacc.Bacc(
        "TRN2",
        target_bir_lowering=False,
        debug=False,
        enable_asserts=False,
        num_devices=NC,
    )
    xeT_d = nc.dram_tensor("xeT", [E, S], f16, kind="ExternalInput").ap()
    xdT_d = nc.dram_tensor("xdT", [E, S], f16, kind="ExternalInput").ap()
    wqkT_d = nc.dram_tensor("wqkT", [E, 512], f16, kind="ExternalInput").ap()
    wvT_d = nc.dram_tensor("wvT", [E, 256], f16, kind="ExternalInput").ap()
    woT_d = nc.dram_tensor("woT", [256, E], f16, kind="ExternalInput").ap()
    maskT_d = nc.dram_tensor("maskT", [S, S], f16, kind="ExternalInput").ap()
    outT_d = nc.dram_tensor("outT", [E, S], f16, kind="ExternalOutput").ap()

    with tile.TileContext(nc) as tc, ExitStack() as ctx:
        const = ctx.enter_context(tc.tile_pool(name="const", bufs=1))
        xe_pool = ctx.enter_context(tc.tile_pool(name="xe", bufs=10))
        xd_pool = ctx.enter_context(tc.tile_pool(name="xd", bufs=16))
        ex_pool = ctx.enter_context(tc.tile_pool(name="ex", bufs=20))
        out_pool = ctx.enter_context(tc.tile_pool(name="outp", bufs=3))
        div_pool = ctx.enter_context(tc.tile_pool(name="divp", bufs=2))
        qk_ps = ctx.enter_context(tc.tile_pool(name="qkps", bufs=2, space="PSUM"))
        pv_ps = ctx.enter_context(tc.tile_pool(name="pvps", bufs=4, space="PSUM"))

        # ---- static SBUF tensors -----------------------------------------
        exp_bias = const.tile([128, 1], f32, name="exp_bias")
        nc.vector.memset(exp_bias[:], EXP_SHIFT)
        ones_sb = const.tile([65, 64], f16, name="ones_sb")
        nc.vector.memset(ones_sb[:], 1.0)

        wqk_sb = []
        for es in range(ES):
            wq = const.tile([128, 512], f16, name=f"wqk{es}")
            nc.sync.dma_start(wq[:], wqkT_d[es * 128 : (es + 1) * 128, :])
            wqk_sb.append(wq)
        wv_sb = []
        for es in range(ES):
            wv = const.tile([128, 256], f16, name=f"wv{es}")
            nc.sync.dma_start(wv[:], wvT_d[es * 128 : (es + 1) * 128, :])
            wv_sb.append(wv)
        wo_sb = []
        for hp in range(2):
            wo = const.tile([128, E], f16, name=f"wo{hp}")
            nc.sync.dma_start(wo[:], woT_d[hp * 128 : (hp + 1) * 128, :])
            wo_sb.append(wo)

        mask_sb = [const.tile([128, S], f16, name=f"mask{jt}") for jt in range(JT)]

        def emit_mask_chunk(ib, jt, nib=1):
            # on the gpsimd DMA queue: runs in parallel with the sync-queue
            # input streams (xe/xd/weights), keeping phase B off the mask's
            # bandwidth
            nc.gpsimd.dma_start(
                mask_sb[jt][:, ib * 512 : (ib + nib) * 512],
                maskT_d[jt * 128 : (jt + 1) * 128, ib * 512 : (ib + nib) * 512],
            )

        k_sb = []
        q_sb = []
        vals_sb = []
        for hp in range(2):
            k_sb.append(const.tile([128, S], f16, name=f"ksb{hp}"))
            q_sb.append(const.tile([128, S], f16, name=f"qsb{hp}"))
            vals_sb.append(const.tile([128, S], f16, name=f"valssb{hp}"))
        v_sb = [const.tile([128, 4 * 65], f16, name=f"vsb{jt}") for jt in range(JT)]

        # xd tiles for phase C, loaded in 4 groups of 8 (one per st-quad)
        xd_t = [[None] * ES for _ in range(4)]

        def emit_xd_group(stq):
            for es in range(ES):
                t = xd_pool.tile([128, 512], f16, name="xdt")
                nc.sync.dma_start(
                    t[:],
                    xdT_d[es * 128 : (es + 1) * 128, stq * 512 : stq * 512 + 512],
                )
                xd_t[stq][es] = t

        # ---- phase B: q/k projection -------------------------------------
        # qkT[c, s] = sum_e wqkT[e, c] * xeT[e, s]; chan tiles:
        #   ct0=[k_h0;k_h1] ct1=[q_h0;q_h1] ct2=[k_h2;k_h3] ct3=[q_h2;q_h3]
        xe_t = [[None] * ES, [None] * ES]  # per sb_i-pair
        for sb_i in range(4):
            if sb_i == 2:
                emit_xd_group(0)
            if sb_i == 3:
                emit_xd_group(1)
            pa = qk_ps.tile([128, 1024], f32, name="projps_a", tag="qkps")
            pb = qk_ps.tile([128, 1024], f32, name="projps_b", tag="qkps")
            halves = [pa[:, 0:512], pa[:, 512:1024], pb[:, 0:512], pb[:, 512:1024]]
            for es in range(ES):
                if sb_i % 2 == 0:
                    t = xe_pool.tile([128, 1024], f16, name="xet")
                    nc.sync.dma_start(
                        t[:],
                        xeT_d[
                            es * 128 : (es + 1) * 128,
                            sb_i * 512 : sb_i * 512 + 1024,
                        ],
                    )
                    xe_t[sb_i // 2][es] = t
                xrhs = xe_t[sb_i // 2][es][:, (sb_i % 2) * 512 : (sb_i % 2) * 512 + 512]
                if es % 2 == 0:
                    emit_mask_chunk(0, sb_i * 4 + es // 2)
                for ct in range(4):
                    nc.tensor.matmul(
                        halves[ct],
                        lhsT=wqk_sb[es][:, ct * 128 : (ct + 1) * 128],
                        rhs=xrhs,
                        start=(es == 0),
                        stop=(es == ES - 1),
                    )
            ssl = slice(sb_i * 512, sb_i * 512 + 512)
            nc.scalar.copy(k_sb[0][:, ssl], pa[:, 0:512])
            nc.scalar.copy(q_sb[0][:, ssl], pa[:, 512:1024])
            nc.scalar.copy(k_sb[1][:, ssl], pb[:, 0:512])
            nc.scalar.copy(q_sb[1][:, ssl], pb[:, 512:1024])

        # ---- shared attention-slot emitters --------------------------------
        def emit_logits_exp_mask(hp, ib, jt):
            """One j-tile of transposed logits + exp + mask; returns ex tile."""
            isl = slice(ib * 512, ib * 512 + 512)
            jsl = slice(jt * 128, jt * 128 + 128)
            qk_t = qk_ps.tile([128, 1024], f32, name="qkt", tag="qkps")
            nc.tensor.matmul(
                qk_t[:, 0:512],
                lhsT=k_sb[hp][0:64, jsl],
                rhs=q_sb[hp][0:64, isl],
                start=True,
                stop=True,
            )
            nc.tensor.matmul(
                qk_t[:, 512:1024],
                lhsT=k_sb[hp][64:128, jsl],
                rhs=q_sb[hp][64:128, isl],
                start=True,
                stop=True,
            )
            ex_t = ex_pool.tile([128, 1024], f16, name="ext")
            nc.scalar.activation(
                ex_t[:],
                qk_t[:],
                mybir.ActivationFunctionType.Exp,
                bias=exp_bias[:],
                scale=NEG_SCALE,
            )
            mview = mask_sb[jt][:, isl].unsqueeze(1).broadcast_to([128, 2, 512])
            nc.vector.tensor_mul(
                ex_t[:].rearrange("p (two c) -> p two c", c=512),
                ex_t[:].rearrange("p (two c) -> p two c", c=512),
                mview,
            )
            return ex_t

        def emit_pv(hp, jt, ex_t, pvA, pvB):
            vview = v_sb[jt].rearrange("p (h c) -> p h c", c=65)
            nc.tensor.matmul(
                pvA[:],
                lhsT=vview[:, 2 * hp, :],
                rhs=ex_t[:, 0:512],
                start=(jt == 0),
                stop=(jt == JT - 1),
            )
            nc.tensor.matmul(
                pvB[:],
                lhsT=vview[:, 2 * hp + 1, :],
                rhs=ex_t[:, 512:1024],
                start=(jt == 0),
                stop=(jt == JT - 1),
            )

        def emit_normalize(hp, ib, pvA, pvB):
            """vals = pv[0:64] / pv[64]; reciprocal broadcast via a 1-row PE
            matmul from partition 64 (no DRAM bounce)."""
            isl = slice(ib * 512, ib * 512 + 512)
            rsAB = div_pool.tile([65, 1024], f16, name="rsAB")
            nc.vector.tensor_copy(rsAB[64:65, 0:512], pvA[64:65, :])
            nc.vector.tensor_copy(rsAB[64:65, 512:1024], pvB[64:65, :])
            rb_ps = qk_ps.tile([64, 1024], f32, name="rbps", tag="qkps")
            nc.tensor.matmul(
                rb_ps[:, 0:512],
                lhsT=ones_sb[64:65, :],
                rhs=rsAB[64:65, 0:512],
                start=True,
                stop=True,
            )
            nc.tensor.matmul(
                rb_ps[:, 512:1024],
                lhsT=ones_sb[64:65, :],
                rhs=rsAB[64:65, 512:1024],
                start=True,
                stop=True,
            )
            rc = div_pool.tile([64, 1024], f32, name="rc")
            nc.vector.reciprocal_approx_fast(rc[:], rb_ps[:])
            nc.vector.tensor_mul(vals_sb[hp][0:64, isl], pvA[0:64, :], rc[:, 0:512])
            vtmp = div_pool.tile([64, 512], f16, name="vtmp")
            nc.vector.tensor_mul(vtmp[:], pvB[0:64, :], rc[:, 512:1024])
            nc.sync.dma_start(vals_sb[hp][64:128, isl], vtmp[:])

        def emit_wo(ib, et_range, copies_on_scalar=False):
            isl = slice(ib * 512, ib * 512 + 512)
            for et in et_range:
                po = qk_ps.tile([128, 512], f32, name="wops", tag="qkps")
                for hp2 in range(2):
                    nc.tensor.matmul(
                        po[:],
                        lhsT=wo_sb[hp2][:, et * 128 : (et + 1) * 128],
                        rhs=vals_sb[hp2][:, isl],
                        start=(hp2 == 0),
                        stop=(hp2 == 1),
                    )
                ot = out_pool.tile([128, 512], f16, name="ot")
                if copies_on_scalar:
                    nc.scalar.copy(ot[:], po[:])
                else:
                    nc.vector.tensor_copy(ot[:], po[:])
                nc.sync.dma_start(outT_d[et * 128 : (et + 1) * 128, isl], ot[:])

        # ---- fused loop: logits/exp/mask(hp0, ib0) + phase C (v proj) ------
        ex0 = [None] * JT
        for jt in range(JT):
            if jt == 0:
                emit_xd_group(2)
            if jt == 4:
                emit_xd_group(3)
            ex0[jt] = emit_logits_exp_mask(0, 0, jt)
            emit_mask_chunk(1, jt, nib=2)  # prefetch ib1+ib2 masks
            # C group st=jt
            st = jt
            stq, st_l = st // 4, st % 4
            ssl = slice(st_l * 128, st_l * 128 + 128)
            pv_c = pv_ps.tile([128, 256], f32, name="vprojps", tag="pvps")
            for es in range(ES):
                nc.tensor.matmul(
                    pv_c[:],
                    lhsT=xd_t[stq][es][:, ssl],
                    rhs=wv_sb[es][:],
                    start=(es == 0),
                    stop=(es == ES - 1),
                )
            view = v_sb[st].rearrange("p (h c) -> p h c", c=65)
            nc.vector.tensor_copy(
                view[:, :, 0:64], pv_c[:].rearrange("p (h c) -> p h c", c=64)
            )
            nc.vector.memset(view[:, :, 64:65], 1.0)

        # ---- attention blocks ----------------------------------------------
        # blocks in order; (0,0)'s PV is lagged into the (0,1) block.
        blocks = [(0, 1), (0, 2), (0, 3), (1, 0), (1, 1), (1, 2), (1, 3)]
        pend_norm = None  # (hp, ib, pvA, pvB) awaiting normalize
        pend_wo = None  # ib awaiting W_o emission (hp==1 blocks only)
        pv00A = pv00B = None  # lagged (0,0) accumulators

        for bi, (hp, ib) in enumerate(blocks):
            first = bi == 0
            # prefetch mask chunks ahead (this block's own chunks were
            # loaded at least one block earlier)
            mask_ib = {0: 3}.get(bi)  # ib3 during block (0,1)
            pvA = pv_ps.tile([65, 512], f32, name="pvA", tag="pvps")
            pvB = pv_ps.tile([65, 512], f32, name="pvB", tag="pvps")
            if first:
                pv00A = pv_ps.tile([65, 512], f32, name="pv0A", tag="pvps")
                pv00B = pv_ps.tile([65, 512], f32, name="pv0B", tag="pvps")
            for jt in range(JT):
                ex_t = emit_logits_exp_mask(hp, ib, jt)
                if first:
                    emit_pv(0, jt, ex0[jt], pv00A, pv00B)
                emit_pv(hp, jt, ex_t, pvA, pvB)
                if mask_ib is not None:
                    emit_mask_chunk(mask_ib, jt)
                if jt == 2 and pend_norm is not None:
                    emit_normalize(*pend_norm)
                    pend_norm = None
                # W_o of the previous i-block in shallow 2-et groups so the
                # in-order PE/DVE queues never build a deep backlog
                if jt in (4, 7, 10, 13) and pend_wo is not None:
                    emit_wo(pend_wo, range((jt - 4) // 3 * 2, (jt - 4) // 3 * 2 + 2))
                    if jt == 13:
                        pend_wo = None
            if first:
                emit_normalize(0, 0, pv00A, pv00B)
            pend_norm = (hp, ib, pvA, pvB)
            if hp == 1:
                pend_wo = ib

        # tail: last block's normalize + W_o
        emit_normalize(*pend_norm)
        emit_wo(pend_wo, range(0, 8), copies_on_scalar=True)

    nc.compile()
    return nc


_NC_CACHE = None


def _get_nc():
    global _NC_CACHE
    if _NC_CACHE is None:
        _NC_CACHE = build_nc()
    return _NC_CACHE


def shard_inputs(x_encoder, x_decoder, mask, W_qk, W_v, W_o):
    """Build the 8 per-core input maps (fp16 on host)."""
    x_encoder = np.asarray(x_encoder)
    x_decoder = np.asarray(x_decoder)
    mask = np.asarray(mask)
    W_qk = np.asarray(W_qk)
    W_v = np.asarray(W_v)
    W_o = np.asarray(W_o)

    xeT = [
        np.ascontiguousarray(x_encoder[b].T).astype(np.float16)
        for b in range(N_BATCH)
    ]
    xdT = [
        np.ascontiguousarray(x_decoder[b].T).astype(np.float16)
        for b in range(N_BATCH)
    ]
    maskT = np.ascontiguousarray(mask.T).astype(np.float16)

    wqkT_g = []
    wvT_g = []
    woT_g = []
    for g in range(G):
        rows = []
        for hp in range(2):
            h0 = 4 * g + 2 * hp
            h1 = h0 + 1
            # k chans for the pair, then q chans (matches ct order)
            rows.append(W_qk[128 * h0 + 64 : 128 * h0 + 128])
            rows.append(W_qk[128 * h1 + 64 : 128 * h1 + 128])
            rows.append(W_qk[128 * h0 : 128 * h0 + 64])
            rows.append(W_qk[128 * h1 : 128 * h1 + 64])
        sel = np.concatenate(rows, axis=0)
        wqkT_g.append(np.ascontiguousarray(sel.T).astype(np.float16))
        wvT_g.append(
            np.ascontiguousarray(W_v[256 * g : 256 * g + 256, :].T).astype(np.float16)
        )
        woT_g.append(
            np.ascontiguousarray(W_o[:, 256 * g : 256 * g + 256].T).astype(np.float16)
        )

    in_maps = []
    for c in range(NC):
        b, g = c // G, c % G
        in_maps.append(
            {
                "xeT": xeT[b],
                "xdT": xdT[b],
                "wqkT": wqkT_g[g],
                "wvT": wvT_g[g],
                "woT": woT_g[g],
                "maskT": maskT,
            }
        )
    return in_maps


def gather_outputs(results):
    """Sum per-group partials and transpose back to (N, S, E)."""
    out = np.empty((N_BATCH, S, E), dtype=np.float32)
    for b in range(N_BATCH):
        acc = results[b * G]["outT"].astype(np.float32)
        for g in range(1, G):
            acc = acc + results[b * G + g]["outT"].astype(np.float32)
        out[b] = acc.T
    return out


def kernel(x_encoder, x_decoder, mask, W_qk, W_v, W_o):
    nc = _get_nc()
    in_maps = shard_inputs(x_encoder, x_decoder, mask, W_qk, W_v, W_o)
    res = bass_utils.run_bass_kernel_spmd(
        nc, in_maps, core_ids=list(range(NC)), trace=False
    )
    kernel.last_results = res
    return gather_outputs(res.results)
